# revision 40
# baseline (speedup 1.0000x reference)
"""Triangular matmul C = triu(triu(A) @ triu(B)) on 8 TRN2 NeuronCores.

v8 design: the (I, K, J) block-tetrahedron {I <= K <= J} (128x128 blocks,
N=4096 -> 32 blocks/side) is partitioned into PIECES (I, J-window): rows
I < 16 are split at column 2048 into L = [I*128, 2048) and R = [2048, 4096)
halves; rows I >= 16 stay whole (W).  Pieces are assigned to the 8 cores by
a host-side optimizer balancing max(PE time, DMA stream time) -- ~48.5us
each (PE rate ~2 cols/ns measured, stream ~350 B/ns/core measured).

Per core:
- Resident B strips: strip K covers only the column range the core's pieces
  need (L pieces: [128K, 2048); R: [max(128K,2048), 4096); W: full), DMA'd
  once, ascending K, alternating sync/scalar issue queues.
- A^T blocks are packed per-core on the host (apack) in piece-processing
  order and streamed on gpsimd.
- Pieces are grouped into PSUM-feasible batches (sum of banks <= 8) and
  each batch is swept K-MAJOR across its pieces, so the PE consumes strips
  in exactly the order they arrive; bank c of a piece is evicted as soon as
  its last contributing K (min(4c+3, Kmax)) completes.
- Numerics: one bf16 pass (gate is 2e-2; this lands ~4.4e-3), C stored as
  bf16 and upcast on the host.

The kernel takes FULL (unsharded) inputs and returns the FULL output.
"""

import numpy as np

N = 4096
BLK = 128
NB = N // BLK  # 32
N_CORES = 8
PHASE = 512  # PSUM bank width (fp32)
NBANK = 8
NSLOT = 120  # A-pack slots per core (max used: 114)
ACHUNK = 8  # A-load DMA granularity in slots

# Piece tables: ('L', I) covers cols [128I, 2048) of row-block I (I<16);
# ('R', I) covers [2048, 4096) (I<16); ('W', I) covers [128I, 4096) (I>=16).
PIECES = [
    [('L', 15), ('R', 3), ('R', 10), ('R', 11)],
    [('R', 0), ('R', 8)],
    [('R', 4), ('R', 7), ('R', 13), ('W', 27)],
    [('L', 0), ('L', 1), ('L', 2), ('L', 3), ('L', 4), ('L', 5), ('L', 6),
     ('L', 9), ('L', 12), ('L', 13), ('L', 14), ('W', 24), ('W', 29)],
    [('L', 7), ('L', 8), ('L', 10), ('L', 11), ('R', 15), ('W', 17), ('W', 18),
     ('W', 21), ('W', 22), ('W', 23), ('W', 25), ('W', 31)],
    [('R', 9), ('R', 12), ('R', 14), ('W', 19), ('W', 20)],
    [('R', 1), ('R', 2)],
    [('R', 5), ('R', 6), ('W', 16), ('W', 26), ('W', 28), ('W', 30)],
]
MAXP = max(len(pl) for pl in PIECES)  # 13
AHOIST = 64  # A slots loaded unconditionally before the dispatch


def _pdims(p):
    """(u0, u1, Kmax, c0, nbanks) for piece p in 512-col units."""
    t, I = p
    if t == 'L':
        u0, u1, Kmax = I // 4, 4, 15
    elif t == 'R':
        u0, u1, Kmax = 4, 8, 31
    else:
        u0, u1, Kmax = I // 4, 8, 31
    c0 = max(I // 4, u0)
    return u0, u1, Kmax, c0, u1 - c0


def _batches(core):
    """Greedy grouping of pieces (sorted by I) into PSUM-feasible batches."""
    pl = sorted(PIECES[core], key=lambda p: (p[1], p[0]))
    out, cur, banks = [], [], 0
    for p in pl:
        nb = _pdims(p)[4]
        if banks + nb > NBANK and cur:
            out.append(cur)
            cur, banks = [], 0
        cur.append(p)
        banks += nb
    if cur:
        out.append(cur)
    return out


# Flattened processing order per core; slot index = position in this list.
PORDER = [[p for b in _batches(c) for p in b] for c in range(N_CORES)]
# A-pack base slot per piece (piece p occupies Kmax - I + 1 slots).
ABASE = []
for c in range(N_CORES):
    base, d = 0, {}
    for p in PORDER[c]:
        _, _, Kmax, _, _ = _pdims(p)
        d[p] = base
        base += Kmax - p[1] + 1
    assert base <= NSLOT, (c, base)
    ABASE.append(d)


def _strip_ranges(core):
    """Per-strip union column range [s, e) needed by this core's pieces."""
    need = {}
    for p in PORDER[core]:
        t, I = p
        _, _, Kmax, _, _ = _pdims(p)
        for K in range(I, Kmax + 1):
            if t == 'L':
                s, e = K * BLK, 2048
            elif t == 'R':
                s, e = max(K * BLK, 2048), N
            else:
                s, e = K * BLK, N
            if K in need:
                s0, e0 = need[K]
                need[K] = (min(s, s0), max(e, e0))
            else:
                need[K] = (s, e)
    return need


def _emit_loads(nc, tc, pools, dram_io):
    """Pre-dispatch loads: the first AHOIST A slots, split across ALL three
    DMA-capable engines so every ring is warm and still streaming when the
    per-core strip DMAs issue right after the dispatch (a cold ring costs
    ~10us of latency on the first post-dispatch transfer).  gpsimd keeps
    the earliest slots (used first inside the arms)."""
    import concourse.mybir as mybir

    bf16 = mybir.dt.bfloat16
    apool = pools[0]
    apack = dram_io["apack"]

    a_t = apool.tile([BLK, NSLOT, BLK], bf16, name="a_all", tag="a", bufs=1)
    spans = [(nc.gpsimd, 0, 16), (nc.sync, 16, 40), (nc.scalar, 40, AHOIST)]
    for eng, lo, hi in spans:
        for j0 in range(lo, hi, ACHUNK):
            j1 = min(j0 + ACHUNK, hi)
            eng.dma_start(a_t[:, j0:j1, :], apack[:, j0:j1, :])
    return a_t


def _emit_core(nc, tc, pools, dram_io, core, a_t):
    import concourse.mybir as mybir

    f32 = mybir.dt.float32
    bf16 = mybir.dt.bfloat16
    apool, bpool, cpool, psum_pool = pools
    apack, bh, cpart = dram_io["apack"], dram_io["bh"], dram_io["cpart"]

    porder = PORDER[core]
    slot = {p: s for s, p in enumerate(porder)}
    nslots = sum(_pdims(p)[2] - p[1] + 1 for p in porder)

    # --- resident B strips (union ranges), ascending K; the first two are
    # issued in 512-col chunks so the first matmuls start sooner ---
    beng = [nc.sync, nc.scalar]
    ranges = _strip_ranges(core)
    bsb = {}
    ei = 0
    for i, K in enumerate(sorted(ranges)):
        s, e = ranges[K]
        t = bpool.tile([BLK, e - s], bf16, name=f"bs_{K}", tag=f"bs{K}", bufs=1)
        step = PHASE if i < 4 else (e - s)
        for x0 in range(s, e, step):
            x1 = min(x0 + step, e)
            beng[ei % 2].dma_start(
                t[:, x0 - s : x1 - s], bh[K * BLK : (K + 1) * BLK, x0:x1]
            )
            ei += 1
        bsb[K] = (t, s)

    # --- remaining A slots in processing order, gpsimd ---
    for j0 in range(AHOIST, nslots, ACHUNK):
        j1 = min(j0 + ACHUNK, nslots)
        nc.gpsimd.dma_start(a_t[:, j0:j1, :], apack[:, j0:j1, :])

    # --- compute: batches of pieces, K-major within each batch.
    # Eviction copies go ONLY to vector (nothing else queues there) and
    # C stores ONLY to gpsimd, so PSUM-WAR waits resolve at copy latency
    # instead of queuing behind strip-DMA waits on scalar/sync. ---
    ev = 0
    free_order = list(range(NBANK))  # physical tags, earliest-free first
    for batch in _batches(core):
        # assign tags in the previous batch's eviction order: the pieces
        # that join first get the banks that freed earliest
        ps = {}
        used = []
        idx = 0
        for p in batch:
            u0, u1, Kmax, c0, nb = _pdims(p)
            ps[p] = {}
            for cc in range(c0, u1):
                tagid = free_order[idx]
                idx += 1
                ps[p][cc] = psum_pool.tile(
                    [BLK, PHASE], f32, name=f"ps_{core}_{p[0]}{p[1]}_{cc}",
                    tag=f"ps{tagid}",
                )
                used.append((min(4 * cc + 3, Kmax), tagid))
        free_order = free_order[idx:] + [t for _, t in sorted(used)]
        Kmin = min(p[1] for p in batch)
        KmaxB = max(_pdims(p)[2] for p in batch)
        for K in range(Kmin, KmaxB + 1):
            bt, bs = bsb[K][0], bsb[K][1]
            for p in batch:
                t, I = p
                u0, u1, Kmax, c0, nb = _pdims(p)
                if not (I <= K <= Kmax):
                    continue
                a_w = a_t[:, ABASE[core][p] + K - I, :]
                for cc in range(max(c0, K // 4), u1):
                    cstart = max(K * BLK, PHASE * cc)
                    cwidth = PHASE * (cc + 1) - cstart
                    o = ps[p][cc][:, cstart - PHASE * cc : PHASE]
                    nc.tensor.matmul(
                        o, a_w, bt[:, cstart - bs : cstart - bs + cwidth],
                        start=(K == I), stop=(K == min(4 * cc + 3, Kmax)),
                    )
            # evict piece-banks whose last contributing K just ran
            for p in batch:
                t, I = p
                u0, u1, Kmax, c0, nb = _pdims(p)
                if not (I <= K <= Kmax):
                    continue
                for cc in range(c0, u1):
                    if min(4 * cc + 3, Kmax) == K:
                        coff0 = max(I * BLK - PHASE * cc, 0)
                        w = PHASE - coff0
                        ct = cpool.tile(
                            [BLK, PHASE], bf16, name=f"c_{core}_{p[0]}{I}_{cc}",
                            tag="cst",
                        )
                        nc.vector.tensor_copy(ct[:, :w], ps[p][cc][:, coff0:PHASE])
                        r0 = slot[p] * BLK
                        nc.gpsimd.dma_start(
                            cpart[r0 : r0 + BLK, PHASE * cc + coff0 : PHASE * (cc + 1)],
                            ct[:, :w],
                        )
                        ev += 1


def _build():
    import concourse.mybir as mybir
    import concourse.tile as tile
    from concourse import bacc

    nc = bacc.Bacc(None, target_bir_lowering=False, debug=False)
    bf16 = mybir.dt.bfloat16
    with tile.TileContext(nc) as tc:
        with (
            tc.tile_pool(name="dram", bufs=1, space="DRAM") as dram,
            tc.tile_pool(name="apool", bufs=1) as apool,
            tc.tile_pool(name="bpool", bufs=1) as bpool,
            tc.tile_pool(name="cpool", bufs=6) as cpool,
            tc.tile_pool(name="psum", bufs=1, space="PSUM") as psum_pool,
        ):
            dram_io = {
                "apack": dram.tile(
                    [BLK, NSLOT, BLK], bf16, kind="ExternalInput",
                    name="apack", uniquify=False,
                ),
                "bh": dram.tile(
                    [N, N], bf16, kind="ExternalInput", name="bh", uniquify=False,
                ),
                "cpart": dram.tile(
                    [MAXP * BLK, N], bf16, kind="ExternalOutput",
                    name="cpart", uniquify=False,
                ),
            }
            pools = (apool, bpool, cpool, psum_pool)
            pid = nc.partition_id()
            hint = tc.switch_hint(
                {e: pid for e in mybir.ALL_ENGINES}, N_CORES, label="coresw"
            )
            a_t = _emit_loads(nc, tc, pools, dram_io)
            for c in tc.Switch(pid, N_CORES, hint=hint):
                _emit_core(nc, tc, pools, dram_io, c, a_t)
    nc.compile()
    return nc


_cached_nc = None


def _get_nc():
    global _cached_nc
    if _cached_nc is None:
        _cached_nc = _build()
    return _cached_nc


def _host_pack(A, B):
    """Per-core apack (A^T blocks in piece order, bf16) and bh (B, bf16)."""
    import ml_dtypes

    bf16 = ml_dtypes.bfloat16
    ath = np.ascontiguousarray(A.T).astype(bf16)
    bh = np.ascontiguousarray(B.astype(bf16))

    apacks = []
    for c in range(N_CORES):
        ap = np.zeros((BLK, NSLOT, BLK), dtype=bf16)
        for p in PORDER[c]:
            t, I = p
            Kmax = _pdims(p)[2]
            base = ABASE[c][p]
            for j, K in enumerate(range(I, Kmax + 1)):
                ap[:, base + j, :] = ath[
                    K * BLK : (K + 1) * BLK, I * BLK : (I + 1) * BLK
                ]
        apacks.append(ap)
    return apacks, bh


LAST = None  # last BassKernelResults (for test harness introspection)


def kernel(A, B):
    global LAST
    import os

    from concourse.bass_utils import run_bass_kernel_spmd

    A = np.asarray(A, dtype=np.float32)
    B = np.asarray(B, dtype=np.float32)
    nc = _get_nc()
    apacks, bh = _host_pack(A, B)
    in_maps = [{"apack": apacks[c], "bh": bh} for c in range(N_CORES)]
    tkw = {}
    if os.environ.get("KTRACE"):
        tkw["trace"] = True
        tkw["tmpdir"] = os.environ.get("KTRACE_DIR") or None
        tc_env = os.environ.get("KTRACE_CORES")
        if tc_env:
            tkw["trace_cores"] = [int(x) for x in tc_env.split(",")]
    res = run_bass_kernel_spmd(nc, in_maps, core_ids=list(range(N_CORES)), **tkw)
    LAST = res

    C = np.zeros((N, N), dtype=np.float32)
    for c in range(N_CORES):
        cp = res.results[c]["cpart"]
        for s, p in enumerate(PORDER[c]):
            t, I = p
            u0, u1 = _pdims(p)[0], _pdims(p)[1]
            j0 = max(PHASE * u0, I * BLK)
            j1 = PHASE * u1
            C[I * BLK : (I + 1) * BLK, j0:j1] = cp[
                s * BLK : (s + 1) * BLK, j0:j1
            ].astype(np.float32)
    return C


# revision 43
# speedup vs baseline: 1.0702x; 1.0702x over previous
"""Triangular matmul C = triu(triu(A) @ triu(B)) on 8 TRN2 NeuronCores.

v8 design: the (I, K, J) block-tetrahedron {I <= K <= J} (128x128 blocks,
N=4096 -> 32 blocks/side) is partitioned into PIECES (I, J-window): rows
I < 16 are split at column 2048 into L = [I*128, 2048) and R = [2048, 4096)
halves; rows I >= 16 stay whole (W).  Pieces are assigned to the 8 cores by
a host-side optimizer balancing max(PE time, DMA stream time) -- ~48.5us
each (PE rate ~2 cols/ns measured, stream ~350 B/ns/core measured).

Per core:
- Resident B strips: strip K covers only the column range the core's pieces
  need (L pieces: [128K, 2048); R: [max(128K,2048), 4096); W: full), DMA'd
  once, ascending K, alternating sync/scalar issue queues.
- A^T blocks are packed per-core on the host (apack) in piece-processing
  order and streamed on gpsimd.
- Pieces are grouped into PSUM-feasible batches (sum of banks <= 8) and
  each batch is swept K-MAJOR across its pieces, so the PE consumes strips
  in exactly the order they arrive; bank c of a piece is evicted as soon as
  its last contributing K (min(4c+3, Kmax)) completes.
- Numerics: one bf16 pass (gate is 2e-2; this lands ~4.4e-3), C stored as
  bf16 and upcast on the host.

The kernel takes FULL (unsharded) inputs and returns the FULL output.
"""

import numpy as np

N = 4096
BLK = 128
NB = N // BLK  # 32
N_CORES = 8
PHASE = 512  # PSUM bank width (fp32)
NBANK = 8
NSLOT = 120  # A-pack slots per core (max used: 114)
ACHUNK = 8  # A-load DMA granularity in slots

# Piece tables: ('L', I) covers cols [128I, 2048) of row-block I (I<16);
# ('R', I) covers [2048, 4096) (I<16); ('W', I) covers [128I, 4096) (I>=16).
PIECES = [
    [('L', 15), ('R', 3), ('R', 10), ('R', 11)],
    [('R', 0), ('R', 8)],
    [('R', 4), ('R', 7), ('R', 13), ('W', 27)],
    [('L', 0), ('L', 1), ('L', 2), ('L', 3), ('L', 4), ('L', 5), ('L', 6),
     ('L', 9), ('L', 12), ('L', 13), ('L', 14), ('W', 24), ('W', 29)],
    [('L', 7), ('L', 8), ('L', 10), ('L', 11), ('R', 15), ('W', 17), ('W', 18),
     ('W', 21), ('W', 22), ('W', 23), ('W', 25), ('W', 31)],
    [('R', 9), ('R', 12), ('R', 14), ('W', 19), ('W', 20)],
    [('R', 1), ('R', 2)],
    [('R', 5), ('R', 6), ('W', 16), ('W', 26), ('W', 28), ('W', 30)],
]
MAXP = max(len(pl) for pl in PIECES)  # 13
AHOIST = 64  # A slots loaded unconditionally before the dispatch


def _pdims(p):
    """(u0, u1, Kmax, c0, nbanks) for piece p in 512-col units."""
    t, I = p
    if t == 'L':
        u0, u1, Kmax = I // 4, 4, 15
    elif t == 'R':
        u0, u1, Kmax = 4, 8, 31
    else:
        u0, u1, Kmax = I // 4, 8, 31
    c0 = max(I // 4, u0)
    return u0, u1, Kmax, c0, u1 - c0


def _pairs(p):
    """Bank-pair groups [(ca, cb)) of piece p; PSUM is allocated as 4
    double-bank tiles so two banks evict with one copy + one store."""
    u0, u1, Kmax, c0, nb = _pdims(p)
    return [(ca, min(ca + 2, u1)) for ca in range(c0, u1, 2)]


def _batches(core):
    """Greedy grouping of pieces (sorted by I) into PSUM-feasible batches
    (capacity: 4 double-bank pair slots)."""
    pl = sorted(PIECES[core], key=lambda p: (p[1], p[0]))
    out, cur, prs = [], [], 0
    for p in pl:
        np_ = len(_pairs(p))
        if prs + np_ > 4 and cur:
            out.append(cur)
            cur, prs = [], 0
        cur.append(p)
        prs += np_
    if cur:
        out.append(cur)
    return out


# Flattened processing order per core; slot index = position in this list.
PORDER = [[p for b in _batches(c) for p in b] for c in range(N_CORES)]
# A-pack base slot per piece (piece p occupies Kmax - I + 1 slots).
ABASE = []
for c in range(N_CORES):
    base, d = 0, {}
    for p in PORDER[c]:
        _, _, Kmax, _, _ = _pdims(p)
        d[p] = base
        base += Kmax - p[1] + 1
    assert base <= NSLOT, (c, base)
    ABASE.append(d)


def _strip_ranges(core):
    """Per-strip union column range [s, e) needed by this core's pieces."""
    need = {}
    for p in PORDER[core]:
        t, I = p
        _, _, Kmax, _, _ = _pdims(p)
        for K in range(I, Kmax + 1):
            if t == 'L':
                s, e = K * BLK, 2048
            elif t == 'R':
                s, e = max(K * BLK, 2048), N
            else:
                s, e = K * BLK, N
            if K in need:
                s0, e0 = need[K]
                need[K] = (min(s, s0), max(e, e0))
            else:
                need[K] = (s, e)
    return need


def _emit_loads(nc, tc, pools, dram_io):
    """Pre-dispatch loads: the first AHOIST A slots, split across ALL three
    DMA-capable engines so every ring is warm and still streaming when the
    per-core strip DMAs issue right after the dispatch (a cold ring costs
    ~10us of latency on the first post-dispatch transfer).  gpsimd keeps
    the earliest slots (used first inside the arms)."""
    import concourse.mybir as mybir

    bf16 = mybir.dt.bfloat16
    apool = pools[0]
    apack = dram_io["apack"]

    a_t = apool.tile([BLK, NSLOT, BLK], bf16, name="a_all", tag="a", bufs=1)
    spans = [(nc.gpsimd, 0, 16), (nc.sync, 16, 40), (nc.scalar, 40, AHOIST)]
    for eng, lo, hi in spans:
        for j0 in range(lo, hi, ACHUNK):
            j1 = min(j0 + ACHUNK, hi)
            eng.dma_start(a_t[:, j0:j1, :], apack[:, j0:j1, :])
    return a_t


def _emit_core(nc, tc, pools, dram_io, core, a_t):
    import concourse.mybir as mybir

    f32 = mybir.dt.float32
    bf16 = mybir.dt.bfloat16
    apool, bpool, cpool, psum_pool = pools
    apack, bh, cpart = dram_io["apack"], dram_io["bh"], dram_io["cpart"]

    porder = PORDER[core]
    slot = {p: s for s, p in enumerate(porder)}
    nslots = sum(_pdims(p)[2] - p[1] + 1 for p in porder)

    # --- resident B strips (union ranges), ascending K; the first two are
    # issued in 512-col chunks so the first matmuls start sooner ---
    beng = [nc.sync, nc.scalar]
    ranges = _strip_ranges(core)
    bsb = {}
    ei = 0
    for i, K in enumerate(sorted(ranges)):
        s, e = ranges[K]
        t = bpool.tile([BLK, e - s], bf16, name=f"bs_{K}", tag=f"bs{K}", bufs=1)
        step = PHASE if i < 4 else (e - s)
        for x0 in range(s, e, step):
            x1 = min(x0 + step, e)
            beng[ei % 2].dma_start(
                t[:, x0 - s : x1 - s], bh[K * BLK : (K + 1) * BLK, x0:x1]
            )
            ei += 1
        bsb[K] = (t, s)

    # --- remaining A slots in processing order, gpsimd ---
    for j0 in range(AHOIST, nslots, ACHUNK):
        j1 = min(j0 + ACHUNK, nslots)
        nc.gpsimd.dma_start(a_t[:, j0:j1, :], apack[:, j0:j1, :])

    # --- compute: batches of pieces, K-major within each batch.
    # Eviction copies go ONLY to vector (nothing else queues there) and
    # C stores ONLY to gpsimd, so PSUM-WAR waits resolve at copy latency
    # instead of queuing behind strip-DMA waits on scalar/sync. ---
    free_order = list(range(4))  # physical pair-slots, earliest-free first
    for batch in _batches(core):
        # assign pair-slots in the previous batch's eviction order: the
        # pieces that join first get the slots that freed earliest
        ps = {}  # piece -> {bank cc -> (pair tile, ca)}
        pev = {}  # piece -> list of (ca, cb, evictK)
        used = []
        idx = 0
        for p in batch:
            u0, u1, Kmax, c0, nb = _pdims(p)
            ps[p] = {}
            pev[p] = []
            for ca, cb in _pairs(p):
                tagid = free_order[idx]
                idx += 1
                pt = psum_pool.tile(
                    [BLK, PHASE * (cb - ca)], f32,
                    name=f"ps_{core}_{p[0]}{p[1]}_{ca}", tag=f"ps{tagid}",
                )
                evK = min(4 * (cb - 1) + 3, Kmax)
                pev[p].append((ca, cb, evK))
                used.append((evK, tagid))
                for cc in range(ca, cb):
                    ps[p][cc] = (pt, ca)
        free_order = free_order[idx:] + [t for _, t in sorted(used)]
        Kmin = min(p[1] for p in batch)
        KmaxB = max(_pdims(p)[2] for p in batch)
        for K in range(Kmin, KmaxB + 1):
            if K not in bsb:  # no piece in this batch is active at this K
                continue
            bt, bs = bsb[K][0], bsb[K][1]
            for p in batch:
                t, I = p
                u0, u1, Kmax, c0, nb = _pdims(p)
                if not (I <= K <= Kmax):
                    continue
                a_w = a_t[:, ABASE[core][p] + K - I, :]
                for cc in range(max(c0, K // 4), u1):
                    if K > min(4 * cc + 3, Kmax):
                        continue
                    cstart = max(K * BLK, PHASE * cc)
                    cwidth = PHASE * (cc + 1) - cstart
                    pt, ca = ps[p][cc]
                    off = PHASE * (cc - ca)
                    o = pt[:, off + cstart - PHASE * cc : off + PHASE]
                    nc.tensor.matmul(
                        o, a_w, bt[:, cstart - bs : cstart - bs + cwidth],
                        start=(K == I), stop=(K == min(4 * cc + 3, Kmax)),
                    )
            # evict bank-pairs whose last contributing K just ran
            for p in batch:
                t, I = p
                u0, u1, Kmax, c0, nb = _pdims(p)
                if not (I <= K <= Kmax):
                    continue
                for ca, cb, evK in pev[p]:
                    if evK == K:
                        pt = ps[p][ca][0]
                        coff0 = max(I * BLK - PHASE * ca, 0)
                        w = PHASE * (cb - ca) - coff0
                        ct = cpool.tile(
                            [BLK, 2 * PHASE], bf16,
                            name=f"c_{core}_{p[0]}{I}_{ca}", tag="cst",
                        )
                        nc.vector.tensor_copy(
                            ct[:, :w], pt[:, coff0 : PHASE * (cb - ca)]
                        )
                        r0 = slot[p] * BLK
                        nc.gpsimd.dma_start(
                            cpart[r0 : r0 + BLK, PHASE * ca + coff0 : PHASE * cb],
                            ct[:, :w],
                        )


def _build():
    import concourse.mybir as mybir
    import concourse.tile as tile
    from concourse import bacc

    nc = bacc.Bacc(None, target_bir_lowering=False, debug=False)
    bf16 = mybir.dt.bfloat16
    with tile.TileContext(nc) as tc:
        with (
            tc.tile_pool(name="dram", bufs=1, space="DRAM") as dram,
            tc.tile_pool(name="apool", bufs=1) as apool,
            tc.tile_pool(name="bpool", bufs=1) as bpool,
            tc.tile_pool(name="cpool", bufs=6) as cpool,
            tc.tile_pool(name="psum", bufs=1, space="PSUM") as psum_pool,
        ):
            dram_io = {
                "apack": dram.tile(
                    [BLK, NSLOT, BLK], bf16, kind="ExternalInput",
                    name="apack", uniquify=False,
                ),
                "bh": dram.tile(
                    [N, N], bf16, kind="ExternalInput", name="bh", uniquify=False,
                ),
                "cpart": dram.tile(
                    [MAXP * BLK, N], bf16, kind="ExternalOutput",
                    name="cpart", uniquify=False,
                ),
            }
            pools = (apool, bpool, cpool, psum_pool)
            pid = nc.partition_id()
            hint = tc.switch_hint(
                {e: pid for e in mybir.ALL_ENGINES}, N_CORES, label="coresw"
            )
            a_t = _emit_loads(nc, tc, pools, dram_io)
            for c in tc.Switch(pid, N_CORES, hint=hint):
                _emit_core(nc, tc, pools, dram_io, c, a_t)
    nc.compile()
    return nc


_cached_nc = None


def _get_nc():
    global _cached_nc
    if _cached_nc is None:
        _cached_nc = _build()
    return _cached_nc


def _host_pack(A, B):
    """Per-core apack (A^T blocks in piece order, bf16) and bh (B, bf16)."""
    import ml_dtypes

    bf16 = ml_dtypes.bfloat16
    ath = np.ascontiguousarray(A.T).astype(bf16)
    bh = np.ascontiguousarray(B.astype(bf16))

    apacks = []
    for c in range(N_CORES):
        ap = np.zeros((BLK, NSLOT, BLK), dtype=bf16)
        for p in PORDER[c]:
            t, I = p
            Kmax = _pdims(p)[2]
            base = ABASE[c][p]
            for j, K in enumerate(range(I, Kmax + 1)):
                ap[:, base + j, :] = ath[
                    K * BLK : (K + 1) * BLK, I * BLK : (I + 1) * BLK
                ]
        apacks.append(ap)
    return apacks, bh


LAST = None  # last BassKernelResults (for test harness introspection)


def kernel(A, B):
    global LAST
    import os

    from concourse.bass_utils import run_bass_kernel_spmd

    A = np.asarray(A, dtype=np.float32)
    B = np.asarray(B, dtype=np.float32)
    nc = _get_nc()
    apacks, bh = _host_pack(A, B)
    in_maps = [{"apack": apacks[c], "bh": bh} for c in range(N_CORES)]
    tkw = {}
    if os.environ.get("KTRACE"):
        tkw["trace"] = True
        tkw["tmpdir"] = os.environ.get("KTRACE_DIR") or None
        tc_env = os.environ.get("KTRACE_CORES")
        if tc_env:
            tkw["trace_cores"] = [int(x) for x in tc_env.split(",")]
    res = run_bass_kernel_spmd(nc, in_maps, core_ids=list(range(N_CORES)), **tkw)
    LAST = res

    C = np.zeros((N, N), dtype=np.float32)
    for c in range(N_CORES):
        cp = res.results[c]["cpart"]
        for s, p in enumerate(PORDER[c]):
            t, I = p
            u0, u1 = _pdims(p)[0], _pdims(p)[1]
            j0 = max(PHASE * u0, I * BLK)
            j1 = PHASE * u1
            C[I * BLK : (I + 1) * BLK, j0:j1] = cp[
                s * BLK : (s + 1) * BLK, j0:j1
            ].astype(np.float32)
    return C
